# revision 15
# baseline (speedup 1.0000x reference)
"""Causal self-attention (B=4, T=2048, C=1024, H=16, D=64) on 8 TRN2 NeuronCores.

Sharding: core c handles batch b=c//2 and head-group g=c%2 (8 of 16 heads,
i.e. 512 of 1024 feature columns). Each core projects q,k,v for its heads,
runs causal softmax(q k^T / sqrt(d)) v, and computes the partial o_proj
attn_out[:, cols] @ Wo[:, cols].T -> [T, C]. Host sums the two head-group
partials per batch and stacks batches.

Kernel structure (streamed over 8 tq-chunks of 256):
  - weights are shipped bf16 (half the startup DMA traffic) and upcast once
    to f32r on the Activation/Vector engines, so every matmul runs in a
    same-dtype pair (f32r x f32r, or bf16 x bf16 for PV).
  - scores are computed transposed per head pair: S^T[tk, tq] (K=64, M=128
    tk, f32r); exp writes bf16 probabilities pt[tk, 2, tq]; diagonal
    128-blocks are masked in place by a gpsimd affine_select.
  - PV runs in the NATURAL orientation: out[tq, d+1] with lhsT = pt (K=tk,
    M=128 tq fully used), rhs = v[tk, 65] (bf16, ones column appended) -
    half the PE cost of the transposed form, and the softmax denominator
    falls out of the same matmul as column 64. Normalization is a
    per-partition broadcast multiply (no DRAM broadcast round-trip).
  - the normalized attn [tq, f] is flipped with PE transposes (f32r
    identity) into attnT [f, tq] for the o_proj matmuls.
  - the score->exp->PV software pipeline runs two steps deep so the
    PE->Act->PE semaphore latency is off the critical path; k/v/q
    projections for chunk c+1 are interleaved into chunk c's exp-paced
    inner loop, and transposes + o_proj groups go into a deferred backlog
    drained preferentially in the late (Activation-bound) chunks.
"""

import numpy as np

B, T, C, H, D = 4, 2048, 1024, 16, 64
NCORES = 8
FH = 512          # features per core = 8 heads
NCT = C // 128    # 8 contraction tiles
JP = 4            # head-pair tiles (8 heads / 2)
CH = 256          # tq chunk width
NCH = T // CH     # 8 chunks

_CACHE = {}


def _build():
    import concourse.bass as bass  # noqa: F401
    import concourse.mybir as mybir
    from concourse import bacc
    from concourse.tile import TileContext

    F32 = mybir.dt.float32
    F32R = mybir.dt.float32r
    BF16 = mybir.dt.bfloat16
    EXP = mybir.ActivationFunctionType.Exp

    nc = bacc.Bacc("TRN2", target_bir_lowering=False, debug=False, num_devices=NCORES)
    xT_h = nc.dram_tensor("xT", [C, T], F32R, kind="ExternalInput")
    wq_h = nc.dram_tensor("wqT", [C, FH], F32R, kind="ExternalInput")
    wk_h = nc.dram_tensor("wkT", [C, FH], F32R, kind="ExternalInput")
    wv_h = nc.dram_tensor("wvT", [C, FH], BF16, kind="ExternalInput")
    wo_h = nc.dram_tensor("woT", [FH, C], BF16, kind="ExternalInput")
    out_h = nc.dram_tensor("out", [T, C], F32, kind="ExternalOutput")
    xT = xT_h.ap()
    out_ap = out_h.ap()

    with TileContext(nc) as tc:
        with (
            tc.tile_pool(name="persist", bufs=1) as persist,
            tc.tile_pool(name="xp", bufs=2) as xp,
            tc.tile_pool(name="qp", bufs=2) as qp,
            tc.tile_pool(name="ptp", bufs=4) as ptp,
            tc.tile_pool(name="wsp", bufs=1) as wsp,
            tc.tile_pool(name="asbp", bufs=2) as asbp,
            tc.tile_pool(name="atp", bufs=5) as atp,
            tc.tile_pool(name="opl", bufs=3) as opool,
            tc.tile_pool(name="rp", bufs=2) as rp,
            tc.tile_pool(name="pvp", bufs=1, space="PSUM") as pvp,
            tc.tile_pool(name="sp", bufs=2, space="PSUM") as sp,
            tc.tile_pool(name="pp", bufs=2, space="PSUM") as pp,
        ):
            wq_s = persist.tile([128, NCT, FH], F32R, tag="wq")
            wk_s = persist.tile([128, NCT, FH], F32R, tag="wk")
            wv_s = persist.tile([128, NCT, FH], F32R, tag="wv")
            wo_s = persist.tile([128, JP, C], BF16, tag="wo")
            kT_s = persist.tile([128, JP, T], F32R, tag="kT")
            v_s = persist.tile([128, 2 * NCH, 8, D + 1], BF16, tag="vs")

            wk_src = wk_h.ap().rearrange("(c p) f -> p c f", p=128)
            wv_src = wv_h.ap().rearrange("(c p) f -> p c f", p=128)
            wq_src = wq_h.ap().rearrange("(c p) f -> p c f", p=128)
            wo_src = wo_h.ap().rearrange("(i p) f -> p i f", p=128)

            def load_xt(c):
                t = xp.tile([128, NCT, CH], F32R, tag="xt")
                src = xT[:, c * CH:(c + 1) * CH].rearrange("(c p) t -> p c t", p=128)
                nc.sync.dma_start(out=t[:, 0:4, :], in_=src[:, 0:4, :])
                nc.sync.dma_start(out=t[:, 4:8, :], in_=src[:, 4:8, :])
                return t

            # startup DMAs in critical-path order on the SP HWDGE FIFO:
            # wk+xt0 gate the k-projection (f32 direct), then wq (f32), xt1;
            # wv lands bf16 and is upcast on Act; wo stays bf16 (o_proj runs
            # bf16 x bf16).
            xt0 = xp.tile([128, NCT, CH], F32R, tag="xt")
            xt0_src = xT[:, 0:CH].rearrange("(c p) t -> p c t", p=128)
            for cc in range(0, NCT, 4):
                nc.sync.dma_start(out=wk_s[:, cc:cc + 4, :], in_=wk_src[:, cc:cc + 4, :])
                nc.sync.dma_start(out=xt0[:, cc:cc + 4, :], in_=xt0_src[:, cc:cc + 4, :])
            nc.sync.dma_start(out=wq_s[:, 0:4, :], in_=wq_src[:, 0:4, :])
            nc.sync.dma_start(out=wq_s[:, 4:8, :], in_=wq_src[:, 4:8, :])
            xt_next = load_xt(1)
            wvb = wsp.tile([128, NCT, FH], BF16, tag="stg")
            nc.sync.dma_start(out=wvb, in_=wv_src)
            nc.scalar.copy(out=wv_s, in_=wvb)
            nc.sync.dma_start(out=wo_s, in_=wo_src)

            # ones column of v for the softmax denominators
            nc.gpsimd.memset(v_s[:, :, :, D:D + 1], 1.0)
            # zero weights: opens each pv PSUM bank as ONE accumulation group
            # (psum start_tensor_calc pending-zeroes the whole 2KB region, so
            # per-head groups in a shared bank must not start separately)
            z128 = persist.tile([128, 128], BF16, tag="z128")
            nc.gpsimd.memset(z128, 0.0)
            # f32r identity for PE transposes
            idn = persist.tile([128, 128], F32, tag="idn")
            nc.gpsimd.memset(idn, 1.0)
            nc.gpsimd.affine_select(
                out=idn, in_=idn, compare_op=mybir.AluOpType.is_ge, fill=0.0,
                base=0, pattern=[[1, 128]], channel_multiplier=-1,
            )
            nc.gpsimd.affine_select(
                out=idn, in_=idn, compare_op=mybir.AluOpType.is_ge, fill=0.0,
                base=0, pattern=[[-1, 128]], channel_multiplier=1,
            )

            def k_steps(c, xt_t):
                # k^T projection for chunk c (writes kT_s window)
                for j in range(JP):
                    ps = pp.tile([128, 512], F32, tag="pp")
                    for cc in range(NCT):
                        nc.tensor.matmul(
                            ps[:, 0:CH], wk_s[:, cc, j * 128:(j + 1) * 128],
                            xt_t[:, cc, :],
                            start=(cc == 0), stop=(cc == NCT - 1), skip_group_check=True,
                        )
                        yield 107
                    nc.vector.tensor_copy(out=kT_s[:, j, c * CH:(c + 1) * CH],
                                          in_=ps[:, 0:CH])
                    yield 0

            def v_steps(c, xt_t):
                for tt in range(2):
                    ps = pp.tile([128, 512], F32, tag="pp")
                    for cc in range(NCT):
                        nc.tensor.matmul(
                            ps, xt_t[:, cc, tt * 128:(tt + 1) * 128], wv_s[:, cc, :],
                            start=(cc == 0), stop=(cc == NCT - 1), skip_group_check=True,
                        )
                        yield 213
                    nc.vector.tensor_copy(
                        out=v_s[:, 2 * c + tt, :, 0:D],
                        in_=ps.rearrange("p (h d) -> p h d", h=8),
                    )
                    yield 0

            def q_steps(c, xt_t, qT_t):
                for j in range(JP):
                    ps = pp.tile([128, 512], F32, tag="pp")
                    for cc in range(NCT):
                        nc.tensor.matmul(
                            ps[:, 0:CH], wq_s[:, cc, j * 128:(j + 1) * 128],
                            xt_t[:, cc, :],
                            start=(cc == 0), stop=(cc == NCT - 1), skip_group_check=True,
                        )
                        yield 107
                    nc.vector.tensor_copy(out=qT_t[:, j, :], in_=ps[:, 0:CH])
                    yield 0

            def o_group(c, at, n, mt):
                po = pp.tile([128, 512], F32, tag="pp")
                for i in range(JP):
                    nc.tensor.matmul(
                        po, at[:, i, mt * 128:(mt + 1) * 128],
                        wo_s[:, i, n * 512:(n + 1) * 512],
                        start=(i == 0), stop=(i == JP - 1), skip_group_check=True,
                    )
                    yield 213
                ot = opool.tile([128, 512], F32, tag="ot")
                nc.vector.tensor_copy(out=ot, in_=po)
                nc.sync.dma_start(
                    out=out_ap[c * CH + mt * 128: c * CH + (mt + 1) * 128,
                               n * 512:(n + 1) * 512],
                    in_=ot,
                )
                yield 0

            def tr_steps(asb2, attnT_t):
                # transpose attn [tq, f] -> attnT [f, tq] (PE, via f32r identity)
                for half in range(2):
                    tp = pp.tile([128, 512], F32, tag="pp")
                    tp = tp.rearrange("p (k t) -> p k t", k=2)
                    for k in range(2):
                        fb = 2 * half + k
                        for a in range(2):
                            nc.tensor.transpose(
                                tp[:, k, a * 128:(a + 1) * 128],
                                asb2[:, a, fb * 128:(fb + 1) * 128], idn,
                            )
                            yield 80
                    nc.vector.tensor_copy(
                        out=attnT_t[:, 2 * half:2 * half + 2, :], in_=tp)
                    yield 0

            def chain(*gens):
                for g in gens:
                    yield from g

            SENT = object()

            # ---- prologue: k, q, v projections for chunk 0 ----
            qT_cur = qp.tile([128, JP, CH], F32R, tag="qT")
            for _ in k_steps(0, xt0):
                pass
            for _ in q_steps(0, xt0, qT_cur):
                pass
            for _ in v_steps(0, xt0):
                pass

            # deferred o_proj work: (chunk, cost_ns, generator) entries, FIFO
            backlog = []
            pending_tr = None

            for c in range(NCH):
                nkt = 2 * c + 2
                if c + 2 < NCH:
                    xt_next2 = load_xt(c + 2)
                if pending_tr is not None:
                    # transposes for chunk c-1 run eagerly: they release the
                    # asb buffer (deferring them deadlocks the in-order PE
                    # stream behind next-chunk PV matmuls)
                    for _ in tr_steps(*pending_tr):
                        pass
                # per-chunk Activation-vs-PE balance (ns): spend backlog in
                # chunks where exp time exceeds the PE's mandatory work
                cols = 4096 * c + 3072
                act_ns = cols * 0.833 + (8 * c + 8) * 160
                mand_ns = (cols + 2080 * c + 1560 + 1072) * 0.4167
                kvq_ns = 10250 if c + 1 < NCH else 0
                slack = act_ns - mand_ns - kvq_ns
                gens = []
                est_ns = 0.0
                # force-pull o_proj groups older than 4 chunks so the attnT
                # ring (bufs=5) can never cycle
                while backlog and backlog[0][0] <= c - 4:
                    _, cost, g = backlog.pop(0)
                    gens.append(g)
                    est_ns += cost
                if c + 1 < NCH:
                    qT_next = qp.tile([128, JP, CH], F32R, tag="qT")
                    gens.append(k_steps(c + 1, xt_next))
                    gens.append(v_steps(c + 1, xt_next))
                    gens.append(q_steps(c + 1, xt_next, qT_next))
                    est_ns += 6826 + 3424
                if slack > 2000:
                    quota = slack - 2000
                    while backlog and quota > 0:
                        _, cost, g = backlog.pop(0)
                        gens.append(g)
                        est_ns += cost
                        quota -= cost
                stream = chain(*gens)
                lead = 4 if c == 0 else 2
                S_c = JP * nkt
                emitted_ns = 0.0
                idx = 0

                # one single-bank PSUM tile per (tq-tile, head-half)
                pv = {(a, x): pvp.tile([128, 4, D + 1], F32, tag=f"pv{a}{x}",
                                       name=f"pv{a}{x}")
                      for a in range(2) for x in range(2)}
                for a in range(2):
                    for x in range(2):
                        nc.tensor.matmul(
                            pv[a, x].rearrange("p h e -> p (h e)"), z128,
                            v_s[:, 0, 4 * x:4 * x + 4, :].rearrange(
                                "p h e -> p (h e)"),
                            start=True, stop=False, skip_group_check=True,
                        )

                def emit_pv(j, kt, pt):
                    for hh in range(2):
                        h = 2 * j + hh
                        x, hx = (0, h) if h < 4 else (1, h - 4)
                        for a in range(2):
                            ig = 2 * c + a
                            if kt <= ig:
                                nc.tensor.matmul(
                                    pv[a, x][:, hx, :], pt[:, hh, a * 128:(a + 1) * 128],
                                    v_s[:, kt, h, :],
                                    start=False, stop=(kt == ig),
                                    skip_group_check=True,
                                )

                pend = []
                for j in range(JP):
                    for kt in range(nkt):
                        c0 = 128 if kt == 2 * c + 1 else 0
                        s = sp.tile([128, 2, CH], F32, tag="s")
                        # diagonal tiles compute the full 256 columns (f32r
                        # below 256 moving columns is quarter-rate, so the
                        # full width costs the same and keeps qT at f32r)
                        nc.tensor.matmul(
                            s[:, 0, :], kT_s[0:64, j, kt * 128:(kt + 1) * 128],
                            qT_cur[0:64, j, :], start=True, stop=True,
                        )
                        nc.tensor.matmul(
                            s[:, 1, :], kT_s[64:128, j, kt * 128:(kt + 1) * 128],
                            qT_cur[64:128, j, :], start=True, stop=True,
                        )
                        pt = ptp.tile([128, 2, CH], BF16, tag="pt")
                        if c0 == 0:
                            nc.scalar.activation(out=pt, in_=s, func=EXP, scale=0.125)
                        else:
                            nc.scalar.activation(out=pt[:, :, c0:CH], in_=s[:, :, c0:CH],
                                                 func=EXP, scale=0.125)
                        if kt >= 2 * c:
                            # zero the upper triangle of the diagonal 128-block
                            nc.gpsimd.affine_select(
                                out=pt[:, :, c0:c0 + 128], in_=pt[:, :, c0:c0 + 128],
                                compare_op=mybir.AluOpType.is_ge, fill=0.0,
                                base=0, pattern=[[0, 2], [1, 128]],
                                channel_multiplier=-1,
                            )
                        idx += 1
                        want = est_ns * max(0, idx - lead) / max(1, S_c - lead)
                        while emitted_ns < want:
                            r = next(stream, SENT)
                            if r is SENT:
                                emitted_ns = float("inf")
                                break
                            emitted_ns += r
                        pend.append((j, kt, pt))
                        if len(pend) > 2:
                            emit_pv(*pend.pop(0))
                for e in pend:
                    emit_pv(*e)
                pend = []
                for _ in stream:
                    pass

                # chunk epilogue: normalize (DVE only), then defer transposes
                # and o_proj into the backlog
                rec = rp.tile([128, 2, 2, 4], F32, tag="rec")
                for a in range(2):
                    for x in range(2):
                        nc.vector.reciprocal(out=rec[:, a, x], in_=pv[a, x][:, :, D])
                asb = asbp.tile([128, 2, 8, D], F32, tag="asb")
                for a in range(2):
                    for x in range(2):
                        nc.vector.tensor_mul(
                            asb[:, a, 4 * x:4 * x + 4], pv[a, x][:, :, 0:D],
                            rec[:, a, x, :, None].broadcast_to([128, 4, D]),
                        )
                asb2 = asb.rearrange("p a h d -> p a (h d)")
                attnT_cur = atp.tile([128, JP, CH], BF16, tag="attnT")
                pending_tr = (asb2, attnT_cur)
                for n in range(2):
                    for mt in range(2):
                        backlog.append((c, 853, o_group(c, attnT_cur, n, mt)))
                if c + 1 < NCH:
                    qT_cur = qT_next
                    if c + 2 < NCH:
                        xt_next = xt_next2

            # tail: transposes for the last chunk, then remaining o_proj
            for _ in tr_steps(*pending_tr):
                pass
            for _, _, g in backlog:
                for _ in g:
                    pass

    nc.compile()
    return nc


def _get_nc():
    if "nc" not in _CACHE:
        _CACHE["nc"] = _build()
    return _CACHE["nc"]


def make_in_maps(x, Wq, Wk, Wv, Wo):
    import ml_dtypes

    bf16 = ml_dtypes.bfloat16
    x = np.asarray(x, dtype=np.float32)
    Wq = np.asarray(Wq, dtype=np.float32)
    Wk = np.asarray(Wk, dtype=np.float32)
    Wv = np.asarray(Wv, dtype=np.float32)
    Wo = np.asarray(Wo, dtype=np.float32)
    in_maps = []
    for core in range(NCORES):
        b, g = core // 2, core % 2
        cols = slice(FH * g, FH * (g + 1))
        in_maps.append({
            "xT": np.ascontiguousarray(x[b].T),
            "wqT": np.ascontiguousarray(Wq.T[:, cols]),
            "wkT": np.ascontiguousarray(Wk.T[:, cols]),
            "wvT": np.ascontiguousarray(Wv.T[:, cols]).astype(bf16),
            "woT": np.ascontiguousarray(Wo.T[cols, :]).astype(bf16),
        })
    return in_maps


def gather_out(parts):
    return np.stack([parts[2 * b] + parts[2 * b + 1] for b in range(B)])


def kernel(x, Wq, Wk, Wv, Wo):
    from concourse.bass_utils import run_bass_kernel_spmd

    nc = _get_nc()
    in_maps = make_in_maps(x, Wq, Wk, Wv, Wo)
    try:
        res = run_bass_kernel_spmd(nc, in_maps, core_ids=list(range(NCORES)))
    except Exception:
        # transient NRT device errors have been observed on this fabric;
        # one retry costs nothing when healthy
        res = run_bass_kernel_spmd(nc, in_maps, core_ids=list(range(NCORES)))
    return gather_out([res.results[c]["out"] for c in range(NCORES)])
